# revision 37
# baseline (speedup 1.0000x reference)
"""GCN layer  out = A_norm @ X @ W.T + b  on 8 Trainium2 NeuronCores.

Math:  out = diag(s) (A+I) diag(s) X W^T + b,   s = 1/sqrt(rowsum(A+I)).

Sharding (1D node partition, row-shard): core d owns rows
R_d = [d*1024, (d+1)*1024).

v2: the deg exchange no longer uses collective_compute (whose ncfw firmware
cold start gates the first AllGather at ~73us).  Instead each core sends its
64*s shard [128, 8] fp32 straight into every peer's SBUF with
remote_dma_broadcast (SWDGE -> SDMA, no CC cores involved):

  - 7 broadcast descriptor preps (one per XOR-delta k=1..7) are generated
    early, during phase 1, while the Q7 sequencer is idle; a single
    trigger_dma fires them once s is computed.  The prep with delta k writes
    the payload at receiver slot k, so all addresses are compile-time
    constant under SPMD: receiver r's slot k holds s for global block r^k.
  - The host packs A^T / X j-blocks in the same XOR order (core r's local
    block b = global block r^b) so the permuted s slots line up with the
    matmul operands; the contraction is permutation-invariant.
  - Receivers wait on a remote semaphore (2 increments per sender, 14
    total) before the Xs scaling reads the s tile.  Kernel-start safety:
    every core clears its semaphore range in the framework preamble and the
    NEFF start is runtime-synchronized, and the sends only fire ~35us into
    the sender's execution, far beyond realistic start skew.

Host-side prep stays pure data movement + RNE rounding (transpose, XOR
block permutation, fp8/bf16 casts); every GCN FLOP runs on device.

Device pipeline per core:
  phase 1: DMA the 8MB fp8 AT shard (A batches first, then X batches on the
           same queue so A lands at full HBM BW); row sums deg = colsum(AT)
           via fp8 DoubleRow PE matmuls with a ones stationary.
  s-prep:  deg [1,1024] -> DRAM bounce -> [128, 8] partition-major;
           64*s = Sqrt(4096/deg); u = s/64 likewise, bounced back to a
           [1,1024] row and broadcast to [128, 1024] for the row scaling.
  exchange: trigger the 7 remote broadcasts + local copy of own slot.
  phase 2: Xs = (64 s_j) X_j -> fp8; H^T = Xs^T @ AT fp8 DoubleRow;
           H^T *= s_i/64 -> bf16; out^T += W^T.T @ H^T (bf16 PE); + b;
           DMA out^T [256, 1024] fp32.
Host gathers out^T shards -> [8192, 256] fp32.
"""

import ml_dtypes
import numpy as np
from contextlib import ExitStack

import concourse.bass as bass
import concourse.bacc as bacc
import concourse.tile as tile
from concourse import mybir
from concourse.bass_utils import run_bass_kernel_spmd

P = 128
N = 8192
NCORES = 8
R = N // NCORES          # rows per core (1024)
F = 256                  # IN_F == OUT_F
NJ = N // P              # j-chunks (64)
f32 = mybir.dt.float32
bf16 = mybir.dt.bfloat16
fp8 = mybir.dt.float8e4


def _build_nc():
    nc = bacc.Bacc()
    ATP = nc.declare_dram_parameter("ATP", [P, NJ * R], fp8, isOutput=False)
    XP = nc.declare_dram_parameter("XP", [P, NJ * F], bf16, isOutput=False)
    WT = nc.declare_dram_parameter("WT", [F, F], bf16, isOutput=False)
    B2 = nc.declare_dram_parameter("B2", [P, 2], f32, isOutput=False)
    IDN = nc.declare_dram_parameter("IDN", [P, P], f32, isOutput=False)
    ESL = nc.declare_dram_parameter("ESL", [8, 8 * P], f32, isOutput=False)
    OUTT = nc.declare_dram_parameter("OUTT", [F, R], f32, isOutput=True)

    cc_warm_in = nc.dram_tensor("cc_warm_in", [1, 8], f32)
    cc_warm_out = nc.dram_tensor("cc_warm_out", [NCORES, 8], f32,
                                 addr_space="Shared")

    rsem = nc.alloc_semaphore("rsem")   # bumped by incoming remote payloads
    lsem = nc.alloc_semaphore("lsem")   # bumped when our sends drain
    tsem = nc.alloc_semaphore("tsem")   # gates the send trigger on s ready

    with tile.TileContext(nc) as tc, ExitStack() as ctx:
        singles = ctx.enter_context(tc.tile_pool(name="singles", bufs=1))
        psum = ctx.enter_context(tc.tile_pool(name="psum", bufs=8, space="PSUM"))

        ones8 = singles.tile([P, 2, P], fp8)
        abig = singles.tile([P, NJ * R], fp8)    # resident fp8 AT, 64KB/part
        xbig = singles.tile([P, NJ * F], bf16)   # X bf16, 32KB/part
        xs8 = singles.tile([P, NJ * F], fp8)     # Xs fp8, 16KB/part
        wt_sb = singles.tile([P, 2 * F], bf16)
        b_sb = singles.tile([P, 2], f32)
        deg_sb = singles.tile([1, R], f32)       # deg, free-axis row
        rec128 = singles.tile([P, 8], f32)       # 1/deg  [p,c]=row c*128+p
        src128 = singles.tile([P, 8], f32)       # 64*s   (broadcast payload)
        u128 = singles.tile([P, 8], f32)         # s/64
        uT_sb = singles.tile([8, P], f32)        # u by 128-chunk rows
        idn_sb = singles.tile([P, P], f32)       # 128x128 identity (PE transp)
        esel = singles.tile([8, 8 * P], f32)     # selector weights: block c
                                                 # = e_c outer ones (K=8)
        ident1 = singles.tile([1, 1], f32)       # PE-transpose identity
        warm11 = singles.tile([1, 1], f32)       # Sqrt act-table preload
        recv = singles.tile([P, NJ], f32)        # 64*s, all blocks (slot k)
        ht = singles.tile([P, 2 * R], bf16)      # H^T as [128f, (fc, i)]
        outsb = singles.tile([P, 2 * R], f32)    # out^T as [128o, (oc, i)]

        # Fire-and-forget warmup collective, first instruction: a NEFF with
        # no collectives gets per-core staggered launches (ms-scale skew,
        # measured), while a CC-bearing NEFF launches all 8 cores in sync.
        # Nothing ever waits on it; the ncfw cold start runs concurrently
        # on the CC cores while phase 1 streams A.
        nc.gpsimd.collective_compute(
            "AllGather", mybir.AluOpType.bypass,
            ins=[cc_warm_in[:]], outs=[cc_warm_out[:]],
            replica_groups=[list(range(NCORES))])

        # ---- remote broadcast prep: desc-gen early, fire later -----------
        # ONE broadcast to all 8 same-device peers (incl. self loopback).
        # Per-peer sends with dummy lanes pace ~2k dummy descriptors per
        # send at ~160ns each and serialize the exchange over ~50us
        # (measured); the all-real-dest broadcast has no dummies.  Every
        # receiver takes sender d's payload at slot d: the out AP offset is
        # partition_id*8, resolved at runtime via SWDGE scalar dynamic
        # offsets.
        for k in range(1, NCORES):
            nc.gpsimd.remote_dma_broadcast(
                out_ap=recv[:, k * 8:(k + 1) * 8],
                in_ap=src128[:, 0:8],
                remote_sem=rsem,
                local_sem=lsem,
                rdests=[(0, k) if i == k else None for i in range(NCORES)],
            )

        nc.vector.memset(ones8, 1.0)
        nc.vector.memset(ident1, 1.0)
        nc.vector.memset(warm11, 1.0)
        # preload the Sqrt activation table off the critical path
        nc.scalar.activation(out=warm11[:], in_=warm11[:],
                             func=mybir.ActivationFunctionType.Sqrt, scale=1.0)

        for fc in range(2):
            nc.scalar.dma_start(out=wt_sb[:, fc * F:(fc + 1) * F],
                                in_=WT[fc * P:(fc + 1) * P, :])
        nc.scalar.dma_start(out=b_sb[:], in_=B2[:])
        nc.scalar.dma_start(out=idn_sb[:], in_=IDN[:])
        nc.scalar.dma_start(out=esel[:], in_=ESL[:])

        deg_ps = [psum.tile([P, 512], f32, tag="mm", name=f"deg_ps{i}")
                  for i in range(2)]

        # ---- phase 1: stream A on the sync queue, row sums on PE ---------
        JBATCH = 8                                # j-chunks per DMA (1MB)
        NT = NJ // 2
        for jb in range(NJ // JBATCH):
            lo, hi = jb * JBATCH * R, (jb + 1) * JBATCH * R
            nc.sync.dma_start(out=abig[:, lo:hi], in_=ATP[:, lo:hi])
            for c in range(JBATCH // 2):
                t = jb * JBATCH // 2 + c
                pair = abig[:, t * 2 * R:(t + 1) * 2 * R].rearrange(
                    "p (c q) -> p c q", c=2)
                for ig in range(2):
                    nc.tensor.matmul(
                        deg_ps[ig][:], ones8[:], pair[:, :, ig * 512:(ig + 1) * 512],
                        start=(t == 0), stop=(t == NT - 1),
                        perf_mode=mybir.MatmulPerfMode.DoubleRow)
        # X streams behind A on the same queue: A keeps full HBM BW, X
        # arrives during the s exchange, ahead of the Xs scaling.
        XBATCH = 8
        for xb in range(NJ // XBATCH):
            lo, hi = xb * XBATCH * F, (xb + 1) * XBATCH * F
            nc.sync.dma_start(out=xbig[:, lo:hi], in_=XP[:, lo:hi])

        # ---- deg -> s (64*s, [p, c] = local row c*128+p) -----------------
        # All compute-engine ops: a DMA here can land on a semaphore lane
        # shared with the bulk A/X streams and falsely wait on them
        # (measured 28us).  PE row-transposes move deg onto partitions.
        nc.vector.tensor_copy(out=deg_sb[0:1, 0:512], in_=deg_ps[0][0:1, :])
        nc.scalar.copy(out=deg_sb[0:1, 512:1024], in_=deg_ps[1][0:1, :])
        tp_ps = psum.tile([P, 8], f32, tag="mm", name="tp_ps")
        for c in range(8):
            nc.tensor.transpose(
                tp_ps[:, c:c + 1], deg_sb[0:1, c * P:(c + 1) * P], ident1[:])
        nc.vector.reciprocal(out=rec128[:], in_=tp_ps[:])
        act_s = nc.scalar.activation(out=src128[:], in_=rec128[:],
                             func=mybir.ActivationFunctionType.Sqrt,
                             scale=4096.0)      # sqrt(4096/deg) = 64*s
        # The trigger is gated on tsem (attached post-Tile, incremented by
        # a NoOp after the Sqrt): Tile does not thread the prep's deferred
        # src128 read onto the trigger for user-synced remote descs and
        # otherwise fires it early (measured).
        nc.vector.tensor_copy(out=recv[:, 0:8], in_=src128[:])
        trig = nc.gpsimd.trigger_dma(count=None)

        # own-row scaling u = s_i/64 -> degb, DMA-free (off critical path):
        # PE-transpose u128 -> [8, 128], then broadcast each row to all 128
        # partitions with a K=1 ones-matmul; degb stays in PSUM and feeds
        # the ht multiplies directly.
        nc.scalar.activation(out=u128[:], in_=rec128[:],
                             func=mybir.ActivationFunctionType.Sqrt,
                             scale=1.0 / 4096.0)  # sqrt(1/(4096 deg)) = s/64
        uT_ps = psum.tile([8, P], f32, tag="mm", name="uT_ps")
        nc.tensor.transpose(uT_ps[:], u128[:], idn_sb[:])
        nc.vector.tensor_copy(out=uT_sb[:], in_=uT_ps[:])
        degb_ps = [psum.tile([P, 512], f32, tag="mm", name=f"degb_ps{i}")
                   for i in range(2)]
        for c in range(8):
            nc.tensor.matmul(
                degb_ps[c // 4][:, (c % 4) * P:(c % 4 + 1) * P],
                esel[:, c * P:(c + 1) * P], uT_sb[:], start=True, stop=True)
        degb_sb = singles.tile([P, R], f32)
        for i in range(2):
            nc.vector.tensor_copy(out=degb_sb[:, i * 512:(i + 1) * 512],
                                  in_=degb_ps[i][:])

        # ---- phase 2: Xs, H^T = Xs^T @ AT, fused epilogue ----------------
        # The rsem arrival wait (2 increments x 7 senders = 14) is attached
        # post-Tile: the scheduler's single-core sim cannot see remote
        # increments and would report a deadlock.
        # Ordering anchor: a throwaway Xs-chunk-0 computed from src128
        # chains the PE H^T matmuls (via xs8) behind the deg transposes in
        # the scheduled PE stream; without it the scheduler may emit H^T
        # first and the in-order PE would deadlock against the rsem gate.
        nc.vector.tensor_scalar_mul(xs8[:, 0:F], xbig[:, 0:F],
                                    src128[:, 0:1])
        xs_insts = []
        for jc in range(NJ):
            xs_insts.append(nc.vector.tensor_scalar_mul(
                xs8[:, jc * F:(jc + 1) * F], xbig[:, jc * F:(jc + 1) * F],
                recv[:, jc:jc + 1]))

        o_ps = [psum.tile([P, 512], f32, tag="mm", name=f"o_ps{i}")
                for i in range(4)]
        for fc in range(2):
            h_ps = [psum.tile([P, 512], f32, tag="mm", name=f"h_ps{fc}_{i}")
                    for i in range(2)]
            for t in range(NT):
                lhs = xs8[:, t * 2 * F:(t + 1) * 2 * F].rearrange(
                    "p (c f) -> p c f", c=2)[:, :, fc * P:(fc + 1) * P]
                rpair = abig[:, t * 2 * R:(t + 1) * 2 * R].rearrange(
                    "p (c q) -> p c q", c=2)
                for ig in range(2):
                    nc.tensor.matmul(
                        h_ps[ig][:], lhs,
                        rpair[:, :, ig * 512:(ig + 1) * 512],
                        start=(t == 0), stop=(t == NT - 1),
                        perf_mode=mybir.MatmulPerfMode.DoubleRow)
            # H^T *= s_i/64 -> bf16, then accumulate this fc into out^T
            for ig in range(2):
                nc.vector.tensor_mul(
                    ht[:, fc * R + ig * 512: fc * R + (ig + 1) * 512],
                    h_ps[ig][:], degb_sb[:, ig * 512:(ig + 1) * 512])
            for oc in range(2):
                lhs = wt_sb[:, fc * F + oc * P: fc * F + (oc + 1) * P]
                for ig in range(2):
                    nc.tensor.matmul(
                        o_ps[oc * 2 + ig][:], lhs,
                        ht[:, fc * R + ig * 512: fc * R + (ig + 1) * 512],
                        start=(fc == 0), stop=(fc == 1))

        for oc in range(2):
            for ig in range(2):
                nc.vector.tensor_scalar_add(
                    outsb[:, oc * R + ig * 512: oc * R + (ig + 1) * 512],
                    o_ps[oc * 2 + ig][:], b_sb[:, oc:oc + 1])
                nc.sync.dma_start(
                    out=OUTT[oc * P:(oc + 1) * P, ig * 512:(ig + 1) * 512],
                    in_=outsb[:, oc * R + ig * 512: oc * R + (ig + 1) * 512])

    # Gate every recv consumer on the remote payload arrivals.  Attached
    # after Tile scheduling; Bacc generate_event_semaphores splits multi-waits.
    for inst in xs_insts:
        inst.wait_op(rsem, 14, "sem-ge", check=False)
    # Trigger fires only after the Sqrt activation that produces the
    # payload has completed.  The activation (and every other candidate
    # producer) already carries the hardware-max sync updates, so splice a
    # NoOp with the tsem increment right after it on the same engine —
    # same-engine in-order completion makes the inc fire post-activation.
    from concourse.bass import create_sync_update
    tup = create_sync_update(tsem, 1)
    act_name = act_s.ins.name
    for f in nc.m.functions:
        for bb in f.blocks:
            for idx, inst in enumerate(bb.instructions):
                if inst.name == act_name:
                    bb.instructions.insert(idx + 1, mybir.InstNoOp(
                        name=f"{act_name}.tseminc",
                        engine=inst.engine,
                        bass_nofuse=True,
                        sync_info=mybir.SyncInfo(on_wait=[], on_update=[tup]),
                    ))
                    break
    trig.wait_op(tsem, 1, "sem-ge", check=False)

    # Bacc defers register allocation / extended-ISA encoding / gpsimd
    # library loads to compile(), which runs from finalize().  The axon
    # run path never finalizes on its own.
    nc.finalize()
    return nc


_NC_CACHE = None


def _get_nc():
    global _NC_CACHE
    if _NC_CACHE is None:
        _NC_CACHE = _build_nc()
    return _NC_CACHE


def _prep_inputs(X, A, W, b):
    X = np.asarray(X, dtype=np.float32)
    A = np.asarray(A, dtype=np.float32)
    W = np.asarray(W, dtype=np.float32)
    b = np.asarray(b, dtype=np.float32)
    WTb = np.ascontiguousarray(W.T).astype(ml_dtypes.bfloat16)  # lhsT layout
    B2 = np.ascontiguousarray(b.reshape(2, P).T)  # B2[p, oc] = b[oc*128 + p]
    X16 = X.astype(ml_dtypes.bfloat16)
    IDN = np.eye(P, dtype=np.float32)
    ESL = np.zeros((8, 8 * P), dtype=np.float32)
    for c in range(8):
        ESL[c, c * P:(c + 1) * P] = 1.0
    idx = np.arange(R)
    qq = np.arange(NJ)
    pp = np.arange(P)
    in_maps = []
    for d in range(NCORES):
        # local chunk q, partition p  ->  global row j; slot order is the
        # sender id, so block q>>3 is global block q>>3 on every core.
        # Within a block the payload layout is [p, c] = row c*128 + p
        # (what the PE transpose of the deg row produces).
        jmap = ((d ^ (qq >> 3))[None, :] * R + (qq & 7)[None, :] * P
                + pp[:, None])                    # [128, 64]
        AT = np.ascontiguousarray(A[d * R:(d + 1) * R, :].T)  # [8192, 1024]
        AT[d * R + idx, idx] += 1.0               # fold in A_hat = A + I
        AT8 = AT.astype(ml_dtypes.float8_e4m3)
        ATP = np.ascontiguousarray(AT8[jmap, :]).reshape(P, NJ * R)
        XPd = np.ascontiguousarray(X16[jmap, :]).reshape(P, NJ * F)
        in_maps.append({"ATP": ATP, "XP": XPd, "WT": WTb, "B2": B2,
                        "IDN": IDN, "ESL": ESL})
    return in_maps


def kernel(X, A, W, b, _trace=False, _trace_cores=None):
    nc = _get_nc()
    in_maps = _prep_inputs(X, A, W, b)
    res = run_bass_kernel_spmd(
        nc, in_maps, list(range(NCORES)), trace=_trace,
        trace_cores=_trace_cores)
    out = np.concatenate(
        [res.results[d]["OUTT"].T for d in range(NCORES)], axis=0)
    if _trace:
        kernel.last_exec_time_ns = res.exec_time_ns
        kernel.last_results = res
    return out.astype(np.float32)


if __name__ == "__main__":
    rng = np.random.default_rng(0)
    X = rng.uniform(size=(N, F)).astype(np.float32)
    A = rng.uniform(size=(N, N)).astype(np.float32)
    W = (rng.uniform(size=(F, F)).astype(np.float32) - 0.5) / 8.0
    b = (rng.uniform(size=(F,)).astype(np.float32) - 0.5) / 8.0
    out = kernel(X, A, W, b)
    A_hat = A + np.eye(N, dtype=np.float32)
    d = 1.0 / np.sqrt(A_hat.sum(1))
    ref = (A_hat * d[:, None] * d[None, :]) @ X @ W.T + b
    err = np.abs(out - ref).max() / np.abs(ref).max()
    print("max rel err vs ref-scale:", err)


# revision 41
# speedup vs baseline: 1.2929x; 1.2929x over previous
"""GCN layer  out = A_norm @ X @ W.T + b  on 8 Trainium2 NeuronCores.

Math:  out = diag(s) (A+I) diag(s) X W^T + b,   s = 1/sqrt(rowsum(A+I)).

Sharding (1D node partition, row-shard): core d owns rows
R_d = [d*1024, (d+1)*1024).

v2: the deg exchange no longer uses collective_compute (whose ncfw firmware
cold start gates the first AllGather at ~73us).  Instead each core sends its
64*s shard [128, 8] fp32 straight into every peer's SBUF with
remote_dma_broadcast (SWDGE -> SDMA, no CC cores involved):

  - 7 broadcast descriptor preps (one per XOR-delta k=1..7) are generated
    early, during phase 1, while the Q7 sequencer is idle; a single
    trigger_dma fires them once s is computed.  The prep with delta k writes
    the payload at receiver slot k, so all addresses are compile-time
    constant under SPMD: receiver r's slot k holds s for global block r^k.
  - The host packs A^T / X j-blocks in the same XOR order (core r's local
    block b = global block r^b) so the permuted s slots line up with the
    matmul operands; the contraction is permutation-invariant.
  - Receivers wait on a remote semaphore (2 increments per sender, 14
    total) before the Xs scaling reads the s tile.  Kernel-start safety:
    every core clears its semaphore range in the framework preamble and the
    NEFF start is runtime-synchronized, and the sends only fire ~35us into
    the sender's execution, far beyond realistic start skew.

Host-side prep stays pure data movement + RNE rounding (transpose, XOR
block permutation, fp8/bf16 casts); every GCN FLOP runs on device.

Device pipeline per core:
  phase 1: DMA the 8MB fp8 AT shard (A batches first, then X batches on the
           same queue so A lands at full HBM BW); row sums deg = colsum(AT)
           via fp8 DoubleRow PE matmuls with a ones stationary.
  s-prep:  deg [1,1024] -> DRAM bounce -> [128, 8] partition-major;
           64*s = Sqrt(4096/deg); u = s/64 likewise, bounced back to a
           [1,1024] row and broadcast to [128, 1024] for the row scaling.
  exchange: trigger the 7 remote broadcasts + local copy of own slot.
  phase 2: Xs = (64 s_j) X_j -> fp8; H^T = Xs^T @ AT fp8 DoubleRow;
           H^T *= s_i/64 -> bf16; out^T += W^T.T @ H^T (bf16 PE); + b;
           DMA out^T [256, 1024] fp32.
Host gathers out^T shards -> [8192, 256] fp32.
"""

import ml_dtypes
import numpy as np
from contextlib import ExitStack

import concourse.bass as bass
import concourse.bacc as bacc
import concourse.tile as tile
from concourse import mybir
from concourse.bass_utils import run_bass_kernel_spmd

P = 128
N = 8192
NCORES = 8
R = N // NCORES          # rows per core (1024)
F = 256                  # IN_F == OUT_F
NJ = N // P              # j-chunks (64)
f32 = mybir.dt.float32
bf16 = mybir.dt.bfloat16
fp8 = mybir.dt.float8e4


def _build_nc():
    nc = bacc.Bacc()
    ATP = nc.declare_dram_parameter("ATP", [P, NJ * R], fp8, isOutput=False)
    XP = nc.declare_dram_parameter("XP", [P, NJ * F], bf16, isOutput=False)
    WT = nc.declare_dram_parameter("WT", [F, F], bf16, isOutput=False)
    B2 = nc.declare_dram_parameter("B2", [P, 2], f32, isOutput=False)
    IDN = nc.declare_dram_parameter("IDN", [P, P], f32, isOutput=False)
    ESL = nc.declare_dram_parameter("ESL", [8, 8 * P], f32, isOutput=False)
    OUTT = nc.declare_dram_parameter("OUTT", [F, R], f32, isOutput=True)

    cc_warm_in = nc.dram_tensor("cc_warm_in", [1, 8], f32)
    cc_warm_out = nc.dram_tensor("cc_warm_out", [NCORES, 8], f32,
                                 addr_space="Shared")

    rsem = nc.alloc_semaphore("rsem")   # bumped by incoming remote payloads
    lsem = nc.alloc_semaphore("lsem")   # bumped when our sends drain
    tsem = nc.alloc_semaphore("tsem")   # gates the send trigger on s ready

    with tile.TileContext(nc) as tc, ExitStack() as ctx:
        singles = ctx.enter_context(tc.tile_pool(name="singles", bufs=1))
        psum = ctx.enter_context(tc.tile_pool(name="psum", bufs=8, space="PSUM"))

        ones8 = singles.tile([P, 2, P], fp8)
        abig = singles.tile([P, NJ * R], fp8)    # resident fp8 AT, 64KB/part
        xbig = singles.tile([P, NJ * F], bf16)   # X bf16, 32KB/part
        xs8 = singles.tile([P, NJ * F], fp8)     # Xs fp8, 16KB/part
        wt_sb = singles.tile([P, 2 * F], bf16)
        b_sb = singles.tile([P, 2], f32)
        deg_sb = singles.tile([1, R], f32)       # deg, free-axis row
        rec128 = singles.tile([P, 8], f32)       # 1/deg  [p,c]=row c*128+p
        src128 = singles.tile([P, 8], f32)       # 64*s   (broadcast payload)
        u128 = singles.tile([P, 8], f32)         # s/64
        uT_sb = singles.tile([8, P], f32)        # u by 128-chunk rows
        idn_sb = singles.tile([P, P], f32)       # 128x128 identity (PE transp)
        esel = singles.tile([8, 8 * P], f32)     # selector weights: block c
                                                 # = e_c outer ones (K=8)
        ident1 = singles.tile([1, 1], f32)       # PE-transpose identity
        warm11 = singles.tile([1, 1], f32)       # Sqrt act-table preload
        recv = singles.tile([P, NJ], f32)        # 64*s, all blocks (slot k)
        ht = singles.tile([P, 2 * R], bf16)      # H^T as [128f, (fc, i)]
        outsb = singles.tile([P, 2 * R], f32)    # out^T as [128o, (oc, i)]

        # Fire-and-forget warmup collective, first instruction: a NEFF with
        # no collectives gets per-core staggered launches (ms-scale skew,
        # measured), while a CC-bearing NEFF launches all 8 cores in sync.
        # Nothing ever waits on it; the ncfw cold start runs concurrently
        # on the CC cores while phase 1 streams A.
        nc.gpsimd.collective_compute(
            "AllGather", mybir.AluOpType.bypass,
            ins=[cc_warm_in[:]], outs=[cc_warm_out[:]],
            replica_groups=[list(range(NCORES))])

        # ---- remote broadcast prep: desc-gen early, fire later -----------
        # ONE broadcast to all 8 same-device peers (incl. self loopback).
        # Per-peer sends with dummy lanes pace ~2k dummy descriptors per
        # send at ~160ns each and serialize the exchange over ~50us
        # (measured); the all-real-dest broadcast has no dummies.  Every
        # receiver takes sender d's payload at slot d: the out AP offset is
        # partition_id*8, resolved at runtime via SWDGE scalar dynamic
        # offsets.
        for k in range(1, NCORES):
            nc.gpsimd.remote_dma_broadcast(
                out_ap=recv[:, k * 8:(k + 1) * 8],
                in_ap=src128[:, 0:8],
                remote_sem=rsem,
                local_sem=lsem,
                rdests=[(0, k) if i == k else None for i in range(NCORES)],
            )

        nc.vector.memset(ones8, 1.0)
        nc.vector.memset(ident1, 1.0)
        nc.vector.memset(warm11, 1.0)
        # preload the Sqrt activation table off the critical path
        nc.scalar.activation(out=warm11[:], in_=warm11[:],
                             func=mybir.ActivationFunctionType.Sqrt, scale=1.0)

        for fc in range(2):
            nc.scalar.dma_start(out=wt_sb[:, fc * F:(fc + 1) * F],
                                in_=WT[fc * P:(fc + 1) * P, :])
        nc.scalar.dma_start(out=b_sb[:], in_=B2[:])
        nc.scalar.dma_start(out=idn_sb[:], in_=IDN[:])
        nc.scalar.dma_start(out=esel[:], in_=ESL[:])

        deg_ps = [psum.tile([P, 512], f32, tag="mm", name=f"deg_ps{i}")
                  for i in range(2)]

        # ---- phase 1: stream A on the sync queue, row sums on PE ---------
        JBATCH = 8                                # j-chunks per DMA (1MB)
        NT = NJ // 2
        for jb in range(NJ // JBATCH):
            lo, hi = jb * JBATCH * R, (jb + 1) * JBATCH * R
            nc.sync.dma_start(out=abig[:, lo:hi], in_=ATP[:, lo:hi])
            for c in range(JBATCH // 2):
                t = jb * JBATCH // 2 + c
                pair = abig[:, t * 2 * R:(t + 1) * 2 * R].rearrange(
                    "p (c q) -> p c q", c=2)
                for ig in range(2):
                    nc.tensor.matmul(
                        deg_ps[ig][:], ones8[:], pair[:, :, ig * 512:(ig + 1) * 512],
                        start=(t == 0), stop=(t == NT - 1),
                        perf_mode=mybir.MatmulPerfMode.DoubleRow)
        # X streams behind A on the same queue: A keeps full HBM BW, X
        # arrives during the s exchange, ahead of the Xs scaling.
        XBATCH = 8
        for xb in range(NJ // XBATCH):
            lo, hi = xb * XBATCH * F, (xb + 1) * XBATCH * F
            nc.sync.dma_start(out=xbig[:, lo:hi], in_=XP[:, lo:hi])

        # ---- deg -> s (64*s, [p, c] = local row c*128+p) -----------------
        # All compute-engine ops: a DMA here can land on a semaphore lane
        # shared with the bulk A/X streams and falsely wait on them
        # (measured 28us).  PE row-transposes move deg onto partitions.
        nc.vector.tensor_copy(out=deg_sb[0:1, 0:512], in_=deg_ps[0][0:1, :])
        nc.scalar.copy(out=deg_sb[0:1, 512:1024], in_=deg_ps[1][0:1, :])
        tp_ps = psum.tile([P, 8], f32, tag="mm", name="tp_ps")
        for c in range(8):
            nc.tensor.transpose(
                tp_ps[:, c:c + 1], deg_sb[0:1, c * P:(c + 1) * P], ident1[:])
        nc.vector.reciprocal(out=rec128[:], in_=tp_ps[:])
        act_s = nc.scalar.activation(out=src128[:], in_=rec128[:],
                             func=mybir.ActivationFunctionType.Sqrt,
                             scale=4096.0)      # sqrt(4096/deg) = 64*s
        # The trigger is gated on tsem (attached post-Tile, incremented by
        # a NoOp after the Sqrt): Tile does not thread the prep's deferred
        # src128 read onto the trigger for user-synced remote descs and
        # otherwise fires it early (measured).
        nc.vector.tensor_copy(out=recv[:, 0:8], in_=src128[:])
        trig = nc.gpsimd.trigger_dma(count=None)

        # own-row scaling u = s_i/64 -> degb, DMA-free (off critical path):
        # PE-transpose u128 -> [8, 128], then broadcast each row to all 128
        # partitions with a K=1 ones-matmul; degb stays in PSUM and feeds
        # the ht multiplies directly.
        nc.scalar.activation(out=u128[:], in_=rec128[:],
                             func=mybir.ActivationFunctionType.Sqrt,
                             scale=1.0 / 4096.0)  # sqrt(1/(4096 deg)) = s/64
        uT_ps = psum.tile([8, P], f32, tag="mm", name="uT_ps")
        nc.tensor.transpose(uT_ps[:], u128[:], idn_sb[:])
        nc.vector.tensor_copy(out=uT_sb[:], in_=uT_ps[:])
        degb_ps = [psum.tile([P, 512], f32, tag="mm", name=f"degb_ps{i}")
                   for i in range(2)]
        for c in range(8):
            nc.tensor.matmul(
                degb_ps[c // 4][:, (c % 4) * P:(c % 4 + 1) * P],
                esel[:, c * P:(c + 1) * P], uT_sb[:], start=True, stop=True)
        degb_sb = singles.tile([P, R], f32)
        for i in range(2):
            nc.vector.tensor_copy(out=degb_sb[:, i * 512:(i + 1) * 512],
                                  in_=degb_ps[i][:])

        # ---- phase 2: Xs, H^T = Xs^T @ AT, fused epilogue ----------------
        # The rsem arrival wait (2 increments x 7 senders = 14) is attached
        # post-Tile: the scheduler's single-core sim cannot see remote
        # increments and would report a deadlock.
        # Ordering anchor: a throwaway Xs-chunk-0 computed from src128
        # chains the PE H^T matmuls (via xs8) behind the deg transposes in
        # the scheduled PE stream; without it the scheduler may emit H^T
        # first and the in-order PE would deadlock against the rsem gate.
        nc.vector.tensor_scalar_mul(xs8[:, 0:F], xbig[:, 0:F],
                                    src128[:, 0:1])
        xs_insts = []
        for jc in range(NJ):
            xs_insts.append(nc.vector.tensor_scalar_mul(
                xs8[:, jc * F:(jc + 1) * F], xbig[:, jc * F:(jc + 1) * F],
                recv[:, jc:jc + 1]))

        o_ps = [psum.tile([P, 512], f32, tag="mm", name=f"o_ps{i}")
                for i in range(4)]
        for fc in range(2):
            h_ps = [psum.tile([P, 512], f32, tag="mm", name=f"h_ps{fc}_{i}")
                    for i in range(2)]
            for t in range(NT):
                lhs = xs8[:, t * 2 * F:(t + 1) * 2 * F].rearrange(
                    "p (c f) -> p c f", c=2)[:, :, fc * P:(fc + 1) * P]
                rpair = abig[:, t * 2 * R:(t + 1) * 2 * R].rearrange(
                    "p (c q) -> p c q", c=2)
                for ig in range(2):
                    nc.tensor.matmul(
                        h_ps[ig][:], lhs,
                        rpair[:, :, ig * 512:(ig + 1) * 512],
                        start=(t == 0), stop=(t == NT - 1),
                        perf_mode=mybir.MatmulPerfMode.DoubleRow)
            # H^T *= s_i/64 -> bf16, then accumulate this fc into out^T
            for ig in range(2):
                nc.vector.tensor_mul(
                    ht[:, fc * R + ig * 512: fc * R + (ig + 1) * 512],
                    h_ps[ig][:], degb_sb[:, ig * 512:(ig + 1) * 512])
            for oc in range(2):
                lhs = wt_sb[:, fc * F + oc * P: fc * F + (oc + 1) * P]
                for ig in range(2):
                    nc.tensor.matmul(
                        o_ps[oc * 2 + ig][:], lhs,
                        ht[:, fc * R + ig * 512: fc * R + (ig + 1) * 512],
                        start=(fc == 0), stop=(fc == 1))

        for oc in range(2):
            for ig in range(2):
                nc.vector.tensor_scalar_add(
                    outsb[:, oc * R + ig * 512: oc * R + (ig + 1) * 512],
                    o_ps[oc * 2 + ig][:], b_sb[:, oc:oc + 1])
                nc.sync.dma_start(
                    out=OUTT[oc * P:(oc + 1) * P, ig * 512:(ig + 1) * 512],
                    in_=outsb[:, oc * R + ig * 512: oc * R + (ig + 1) * 512])

    # Gate every recv consumer on the remote payload arrivals.  Attached
    # after Tile scheduling; Bacc generate_event_semaphores splits multi-waits.
    for inst in xs_insts:
        inst.wait_op(rsem, 14, "sem-ge", check=False)
    # Trigger fires only after the Sqrt activation that produces the
    # payload has completed.  The activation (and every other candidate
    # producer) already carries the hardware-max sync updates, so splice a
    # NoOp with the tsem increment right after it on the same engine —
    # same-engine in-order completion makes the inc fire post-activation.
    from concourse.bass import create_sync_update
    tup = create_sync_update(tsem, 1)
    act_name = act_s.ins.name
    for f in nc.m.functions:
        for bb in f.blocks:
            for idx, inst in enumerate(bb.instructions):
                if inst.name == act_name:
                    bb.instructions.insert(idx + 1, mybir.InstNoOp(
                        name=f"{act_name}.tseminc",
                        engine=inst.engine,
                        bass_nofuse=True,
                        sync_info=mybir.SyncInfo(on_wait=[], on_update=[tup]),
                    ))
                    break
    trig.wait_op(tsem, 1, "sem-ge", check=False)

    # Bacc defers register allocation / extended-ISA encoding / gpsimd
    # library loads to compile(), which runs from finalize().  The axon
    # run path never finalizes on its own.
    nc.finalize()
    return nc


_NC_CACHE = None


def _get_nc():
    global _NC_CACHE
    if _NC_CACHE is None:
        _NC_CACHE = _build_nc()
    return _NC_CACHE


def _prep_inputs(X, A, W, b):
    X = np.asarray(X, dtype=np.float32)
    A = np.asarray(A, dtype=np.float32)
    W = np.asarray(W, dtype=np.float32)
    b = np.asarray(b, dtype=np.float32)
    WTb = np.ascontiguousarray(W.T).astype(ml_dtypes.bfloat16)  # lhsT layout
    B2 = np.ascontiguousarray(b.reshape(2, P).T)  # B2[p, oc] = b[oc*128 + p]
    X16 = X.astype(ml_dtypes.bfloat16)
    IDN = np.eye(P, dtype=np.float32)
    ESL = np.zeros((8, 8 * P), dtype=np.float32)
    for c in range(8):
        ESL[c, c * P:(c + 1) * P] = 1.0
    idx = np.arange(R)
    qq = np.arange(NJ)
    pp = np.arange(P)
    in_maps = []
    for d in range(NCORES):
        # local chunk q, partition p  ->  global row j; slot order is the
        # sender id, so block q>>3 is global block q>>3 on every core.
        # Within a block the payload layout is [p, c] = row c*128 + p
        # (what the PE transpose of the deg row produces).
        jmap = ((d ^ (qq >> 3))[None, :] * R + (qq & 7)[None, :] * P
                + pp[:, None])                    # [128, 64]
        AT = np.ascontiguousarray(A[d * R:(d + 1) * R, :].T)  # [8192, 1024]
        AT[d * R + idx, idx] += 1.0               # fold in A_hat = A + I
        AT8 = AT.astype(ml_dtypes.float8_e4m3)
        ATP = np.ascontiguousarray(AT8[jmap, :]).reshape(P, NJ * R)
        XPd = np.ascontiguousarray(X16[jmap, :]).reshape(P, NJ * F)
        in_maps.append({"ATP": ATP, "XP": XPd, "WT": WTb, "B2": B2,
                        "IDN": IDN, "ESL": ESL})
    return in_maps


def kernel(X, A, W, b, _trace=False, _trace_cores=None):
    nc = _get_nc()
    in_maps = _prep_inputs(X, A, W, b)
    res = run_bass_kernel_spmd(
        nc, in_maps, list(range(NCORES)), trace=_trace,
        trace_cores=_trace_cores)
    out = np.concatenate(
        [res.results[d]["OUTT"].T for d in range(NCORES)], axis=0)
    if _trace:
        kernel.last_exec_time_ns = res.exec_time_ns
        kernel.last_results = res
    return out.astype(np.float32)


if __name__ == "__main__":
    rng = np.random.default_rng(0)
    X = rng.uniform(size=(N, F)).astype(np.float32)
    A = rng.uniform(size=(N, N)).astype(np.float32)
    W = (rng.uniform(size=(F, F)).astype(np.float32) - 0.5) / 8.0
    b = (rng.uniform(size=(F,)).astype(np.float32) - 0.5) / 8.0
    out = kernel(X, A, W, b)
    A_hat = A + np.eye(N, dtype=np.float32)
    d = 1.0 / np.sqrt(A_hat.sum(1))
    ref = (A_hat * d[:, None] * d[None, :]) @ X @ W.T + b
    err = np.abs(out - ref).max() / np.abs(ref).max()
    print("max rel err vs ref-scale:", err)


# revision 44
# speedup vs baseline: 1.3906x; 1.0755x over previous
"""GCN layer  out = A_norm @ X @ W.T + b  on 8 Trainium2 NeuronCores.

Math:  out = diag(s) (A+I) diag(s) X W^T + b,   s = 1/sqrt(rowsum(A+I)).

Sharding (1D node partition, row-shard): core d owns rows
R_d = [d*1024, (d+1)*1024).

The deg/s exchange does not use a data collective: each core sends its
64*s shard [128, 8] fp32 straight into every peer's SBUF with
remote_dma_broadcast (SWDGE -> SDMA descriptor path, no collective
firmware on the data path):

  - 7 single-dest broadcast preps (one per XOR-delta k=1..7) generate
    their descriptors early, during phase 1, while the Q7 sequencer is
    idle; a single trigger_dma fires them once s is computed.  The prep
    with delta k writes the payload at receiver slot k, so all addresses
    are compile-time constant under SPMD: receiver r's slot k holds s for
    global block r^k.
  - The host packs A^T / X j-blocks in the same XOR order (core r's local
    block b = global block r^b) so the permuted s slots line up with the
    matmul operands; the contraction is permutation-invariant.
  - Receivers wait on a remote semaphore (2 increments per sender, 14
    total) before the Xs scaling reads the s tile.  The waits are attached
    post-Tile because the scheduler's single-core sim cannot observe
    remote increments.  Kernel-start safety: every core clears its
    semaphore range in the framework preamble, and the sends only fire
    ~40us into the sender's execution, far beyond the measured start skew.
  - A fire-and-forget warmup AllGather is kept as the first instruction:
    a NEFF without any collective gets per-core staggered launches
    (ms-scale skew, measured); a CC-bearing NEFF launches all 8 cores in
    sync.  Nothing ever waits on it.

Host-side prep stays pure data movement + RNE rounding (transpose, XOR
block permutation, fp8/bf16 casts, identity/selector constants); every
GCN FLOP runs on device.

Device pipeline per core:
  phase 1: DMA the 8MB fp8 AT shard (A batches first, then X batches on
           the same queue so A lands at full HBM BW); row sums
           deg = colsum(AT) via fp8 DoubleRow PE matmuls with a ones
           stationary.
  s-prep:  DMA-free (a DMA here can land on a semaphore lane shared with
           the bulk A/X streams and falsely wait ~28us, measured):
           deg row [1,1024] -> [128, 8] via 8 PE row-transposes;
           64*s = Sqrt(4096/deg); u = s_i/64 likewise, spread to
           [128, 1024] with a PE transpose plus 8 selector matmuls.
  exchange: trigger the 7 remote broadcasts + local copy of own slot 0.
  phase 2: Xs = (64 s_j) X_j -> fp8; H^T = Xs^T @ AT fp8 DoubleRow;
           H^T *= s_i/64 -> bf16; out^T += W^T.T @ H^T (bf16 PE); + b;
           DMA out^T [256, 1024] fp32.
Host gathers out^T shards -> [8192, 256] fp32.

Numerics: fp8 A/Xs operands with fp32 accumulation; bf16 W and H^T for
the output matmul.  Measured vs the fp32 reference: rel-l2 ~2.7e-3.
"""

import ml_dtypes
import numpy as np
from contextlib import ExitStack

import concourse.bass as bass
import concourse.bacc as bacc
import concourse.tile as tile
from concourse import mybir
from concourse.bass_utils import run_bass_kernel_spmd

P = 128
N = 8192
NCORES = 8
R = N // NCORES          # rows per core (1024)
F = 256                  # IN_F == OUT_F
NJ = N // P              # j-chunks (64)
f32 = mybir.dt.float32
bf16 = mybir.dt.bfloat16
fp8 = mybir.dt.float8e4


def _build_nc():
    nc = bacc.Bacc()
    ATP = nc.declare_dram_parameter("ATP", [P, NJ * R], fp8, isOutput=False)
    XP = nc.declare_dram_parameter("XP", [P, NJ * F], bf16, isOutput=False)
    WT = nc.declare_dram_parameter("WT", [F, F], bf16, isOutput=False)
    B2 = nc.declare_dram_parameter("B2", [P, 2], f32, isOutput=False)
    IDN = nc.declare_dram_parameter("IDN", [P, P], f32, isOutput=False)
    ESL = nc.declare_dram_parameter("ESL", [8, 8 * P], f32, isOutput=False)
    OUTT = nc.declare_dram_parameter("OUTT", [F, R], f32, isOutput=True)

    cc_warm_in = nc.dram_tensor("cc_warm_in", [1, 8], f32)
    cc_warm_out = nc.dram_tensor("cc_warm_out", [NCORES, 8], f32,
                                 addr_space="Shared")

    rsem = nc.alloc_semaphore("rsem")   # bumped by incoming remote payloads
    lsem = nc.alloc_semaphore("lsem")   # bumped when our sends drain
    tsem = nc.alloc_semaphore("tsem")   # gates the send trigger on s ready

    with tile.TileContext(nc) as tc, ExitStack() as ctx:
        singles = ctx.enter_context(tc.tile_pool(name="singles", bufs=1))
        psum = ctx.enter_context(tc.tile_pool(name="psum", bufs=8, space="PSUM"))

        ones8 = singles.tile([P, 2, P], fp8)
        abig = singles.tile([P, NJ * R], fp8)    # resident fp8 AT, 64KB/part
        xbig = singles.tile([P, NJ * F], bf16)   # X bf16, 32KB/part
        xs8 = singles.tile([P, NJ * F], fp8)     # Xs fp8, 16KB/part
        wt_sb = singles.tile([P, 2 * F], bf16)
        b_sb = singles.tile([P, 2], f32)
        deg_sb = singles.tile([1, R], f32)       # deg, free-axis row
        rec128 = singles.tile([P, 8], f32)       # 1/deg  [p,c]=row c*128+p
        src128 = singles.tile([P, 8], f32)       # 64*s   (broadcast payload)
        u128 = singles.tile([P, 8], f32)         # s/64
        uT_sb = singles.tile([8, P], f32)        # u by 128-chunk rows
        idn_sb = singles.tile([P, P], f32)       # 128x128 identity (PE transp)
        esel = singles.tile([8, 8 * P], f32)     # selector weights: block c
                                                 # = e_c outer ones (K=8)
        ident1 = singles.tile([1, 1], f32)       # PE-transpose identity
        warm11 = singles.tile([1, 1], f32)       # Sqrt act-table preload
        recv = singles.tile([P, NJ], f32)        # 64*s, all blocks (slot k)
        ht = singles.tile([P, 2 * R], bf16)      # H^T as [128f, (fc, i)]
        outsb = singles.tile([P, 2 * R], f32)    # out^T as [128o, (oc, i)]

        # Fire-and-forget warmup collective, first instruction: a NEFF with
        # no collectives gets per-core staggered launches (ms-scale skew,
        # measured), while a CC-bearing NEFF launches all 8 cores in sync.
        # Nothing ever waits on it; the ncfw cold start runs concurrently
        # on the CC cores while phase 1 streams A.
        nc.gpsimd.collective_compute(
            "AllGather", mybir.AluOpType.bypass,
            ins=[cc_warm_in[:]], outs=[cc_warm_out[:]],
            replica_groups=[list(range(NCORES))])

        # ---- remote broadcast prep: desc-gen early, fire later -----------
        # ONE broadcast to all 8 same-device peers (incl. self loopback).
        # Per-peer sends with dummy lanes pace ~2k dummy descriptors per
        # send at ~160ns each and serialize the exchange over ~50us
        # (measured); the all-real-dest broadcast has no dummies.  Every
        # receiver takes sender d's payload at slot d: the out AP offset is
        # partition_id*8, resolved at runtime via SWDGE scalar dynamic
        # offsets.
        for k in range(1, NCORES):
            nc.gpsimd.remote_dma_broadcast(
                out_ap=recv[:, k * 8:(k + 1) * 8],
                in_ap=src128[:, 0:8],
                remote_sem=rsem,
                local_sem=lsem,
                rdests=[(0, k) if i == k else None for i in range(NCORES)],
            )

        nc.vector.memset(ones8, 1.0)
        nc.vector.memset(ident1, 1.0)
        nc.vector.memset(warm11, 1.0)
        # preload the Sqrt activation table off the critical path
        nc.scalar.activation(out=warm11[:], in_=warm11[:],
                             func=mybir.ActivationFunctionType.Sqrt, scale=1.0)

        for fc in range(2):
            nc.scalar.dma_start(out=wt_sb[:, fc * F:(fc + 1) * F],
                                in_=WT[fc * P:(fc + 1) * P, :])
        nc.scalar.dma_start(out=b_sb[:], in_=B2[:])
        nc.scalar.dma_start(out=idn_sb[:], in_=IDN[:])
        nc.scalar.dma_start(out=esel[:], in_=ESL[:])

        deg_ps = [psum.tile([P, 512], f32, tag="mm", name=f"deg_ps{i}")
                  for i in range(2)]

        # ---- phase 1: stream A on the sync queue, row sums on PE ---------
        JBATCH = 8                                # j-chunks per DMA (1MB)
        NT = NJ // 2
        for jb in range(NJ // JBATCH):
            lo, hi = jb * JBATCH * R, (jb + 1) * JBATCH * R
            nc.sync.dma_start(out=abig[:, lo:hi], in_=ATP[:, lo:hi])
            for c in range(JBATCH // 2):
                t = jb * JBATCH // 2 + c
                pair = abig[:, t * 2 * R:(t + 1) * 2 * R].rearrange(
                    "p (c q) -> p c q", c=2)
                for ig in range(2):
                    nc.tensor.matmul(
                        deg_ps[ig][:], ones8[:], pair[:, :, ig * 512:(ig + 1) * 512],
                        start=(t == 0), stop=(t == NT - 1),
                        perf_mode=mybir.MatmulPerfMode.DoubleRow)
        # X streams behind A on the same queue: A keeps full HBM BW, X
        # arrives during the s exchange, ahead of the Xs scaling.
        XBATCH = 8
        for xb in range(NJ // XBATCH):
            lo, hi = xb * XBATCH * F, (xb + 1) * XBATCH * F
            nc.sync.dma_start(out=xbig[:, lo:hi], in_=XP[:, lo:hi])

        # ---- deg -> s (64*s, [p, c] = local row c*128+p) -----------------
        # All compute-engine ops: a DMA here can land on a semaphore lane
        # shared with the bulk A/X streams and falsely wait on them
        # (measured 28us).  PE row-transposes move deg onto partitions.
        nc.vector.tensor_copy(out=deg_sb[0:1, 0:512], in_=deg_ps[0][0:1, :])
        nc.scalar.copy(out=deg_sb[0:1, 512:1024], in_=deg_ps[1][0:1, :])
        tp_ps = psum.tile([P, 8], f32, tag="mm", name="tp_ps")
        for c in range(8):
            nc.tensor.transpose(
                tp_ps[:, c:c + 1], deg_sb[0:1, c * P:(c + 1) * P], ident1[:])
        nc.vector.reciprocal(out=rec128[:], in_=tp_ps[:])
        act_s = nc.scalar.activation(out=src128[:], in_=rec128[:],
                             func=mybir.ActivationFunctionType.Sqrt,
                             scale=4096.0)      # sqrt(4096/deg) = 64*s
        # The trigger is gated on tsem (attached post-Tile, incremented by
        # a NoOp after the Sqrt): Tile does not thread the prep's deferred
        # src128 read onto the trigger for user-synced remote descs and
        # otherwise fires it early (measured).
        nc.vector.tensor_copy(out=recv[:, 0:8], in_=src128[:])
        trig = nc.gpsimd.trigger_dma(count=None)

        # own-row scaling u = s_i/64 -> degb, DMA-free (off critical path):
        # PE-transpose u128 -> [8, 128], then broadcast each row to all 128
        # partitions with a K=1 ones-matmul; degb stays in PSUM and feeds
        # the ht multiplies directly.
        nc.scalar.activation(out=u128[:], in_=rec128[:],
                             func=mybir.ActivationFunctionType.Sqrt,
                             scale=1.0 / 4096.0)  # sqrt(1/(4096 deg)) = s/64
        uT_ps = psum.tile([8, P], f32, tag="mm", name="uT_ps")
        nc.tensor.transpose(uT_ps[:], u128[:], idn_sb[:])
        nc.vector.tensor_copy(out=uT_sb[:], in_=uT_ps[:])
        degb_ps = [psum.tile([P, 512], f32, tag="mm", name=f"degb_ps{i}")
                   for i in range(2)]
        for c in range(8):
            nc.tensor.matmul(
                degb_ps[c // 4][:, (c % 4) * P:(c % 4 + 1) * P],
                esel[:, c * P:(c + 1) * P], uT_sb[:], start=True, stop=True)
        degb_sb = singles.tile([P, R], f32)
        for i in range(2):
            nc.vector.tensor_copy(out=degb_sb[:, i * 512:(i + 1) * 512],
                                  in_=degb_ps[i][:])

        # ---- phase 2: Xs, H^T = Xs^T @ AT, fused epilogue ----------------
        # The rsem arrival wait (2 increments x 7 senders = 14) is attached
        # post-Tile: the scheduler's single-core sim cannot see remote
        # increments and would report a deadlock.
        # Ordering anchor: a throwaway Xs-chunk-0 computed from src128
        # chains the PE H^T matmuls (via xs8) behind the deg transposes in
        # the scheduled PE stream; without it the scheduler may emit H^T
        # first and the in-order PE would deadlock against the rsem gate.
        nc.vector.tensor_scalar_mul(xs8[:, 0:F], xbig[:, 0:F],
                                    src128[:, 0:1])
        xs_insts = []
        for jc in range(NJ):
            xs_insts.append(nc.vector.tensor_scalar_mul(
                xs8[:, jc * F:(jc + 1) * F], xbig[:, jc * F:(jc + 1) * F],
                recv[:, jc:jc + 1]))

        o_ps = [psum.tile([P, 512], f32, tag="mm", name=f"o_ps{i}")
                for i in range(4)]
        for fc in range(2):
            h_ps = [psum.tile([P, 512], f32, tag="mm", name=f"h_ps{fc}_{i}")
                    for i in range(2)]
            for t in range(NT):
                lhs = xs8[:, t * 2 * F:(t + 1) * 2 * F].rearrange(
                    "p (c f) -> p c f", c=2)[:, :, fc * P:(fc + 1) * P]
                rpair = abig[:, t * 2 * R:(t + 1) * 2 * R].rearrange(
                    "p (c q) -> p c q", c=2)
                for ig in range(2):
                    nc.tensor.matmul(
                        h_ps[ig][:], lhs,
                        rpair[:, :, ig * 512:(ig + 1) * 512],
                        start=(t == 0), stop=(t == NT - 1),
                        perf_mode=mybir.MatmulPerfMode.DoubleRow)
            # H^T *= s_i/64 -> bf16, then accumulate this fc into out^T
            for ig in range(2):
                nc.vector.tensor_mul(
                    ht[:, fc * R + ig * 512: fc * R + (ig + 1) * 512],
                    h_ps[ig][:], degb_sb[:, ig * 512:(ig + 1) * 512])
            for oc in range(2):
                lhs = wt_sb[:, fc * F + oc * P: fc * F + (oc + 1) * P]
                for ig in range(2):
                    nc.tensor.matmul(
                        o_ps[oc * 2 + ig][:], lhs,
                        ht[:, fc * R + ig * 512: fc * R + (ig + 1) * 512],
                        start=(fc == 0), stop=(fc == 1))

        for oc in range(2):
            for ig in range(2):
                nc.vector.tensor_scalar_add(
                    outsb[:, oc * R + ig * 512: oc * R + (ig + 1) * 512],
                    o_ps[oc * 2 + ig][:], b_sb[:, oc:oc + 1])
                nc.sync.dma_start(
                    out=OUTT[oc * P:(oc + 1) * P, ig * 512:(ig + 1) * 512],
                    in_=outsb[:, oc * R + ig * 512: oc * R + (ig + 1) * 512])

    # Gate every recv consumer on the remote payload arrivals.  Attached
    # after Tile scheduling; Bacc generate_event_semaphores splits multi-waits.
    for inst in xs_insts:
        inst.wait_op(rsem, 14, "sem-ge", check=False)
    # Trigger fires only after the Sqrt activation that produces the
    # payload has completed.  The activation (and every other candidate
    # producer) already carries the hardware-max sync updates, so splice a
    # NoOp with the tsem increment right after it on the same engine —
    # same-engine in-order completion makes the inc fire post-activation.
    from concourse.bass import create_sync_update
    tup = create_sync_update(tsem, 1)
    act_name = act_s.ins.name
    for f in nc.m.functions:
        for bb in f.blocks:
            for idx, inst in enumerate(bb.instructions):
                if inst.name == act_name:
                    bb.instructions.insert(idx + 1, mybir.InstNoOp(
                        name=f"{act_name}.tseminc",
                        engine=inst.engine,
                        bass_nofuse=True,
                        sync_info=mybir.SyncInfo(on_wait=[], on_update=[tup]),
                    ))
                    break
    trig.wait_op(tsem, 1, "sem-ge", check=False)

    # Bacc defers register allocation / extended-ISA encoding / gpsimd
    # library loads to compile(), which runs from finalize().  The axon
    # run path never finalizes on its own.
    nc.finalize()
    return nc


_NC_CACHE = None


def _get_nc():
    global _NC_CACHE
    if _NC_CACHE is None:
        _NC_CACHE = _build_nc()
    return _NC_CACHE


def _prep_inputs(X, A, W, b):
    X = np.asarray(X, dtype=np.float32)
    A = np.asarray(A, dtype=np.float32)
    W = np.asarray(W, dtype=np.float32)
    b = np.asarray(b, dtype=np.float32)
    WTb = np.ascontiguousarray(W.T).astype(ml_dtypes.bfloat16)  # lhsT layout
    B2 = np.ascontiguousarray(b.reshape(2, P).T)  # B2[p, oc] = b[oc*128 + p]
    X16 = X.astype(ml_dtypes.bfloat16)
    IDN = np.eye(P, dtype=np.float32)
    ESL = np.zeros((8, 8 * P), dtype=np.float32)
    for c in range(8):
        ESL[c, c * P:(c + 1) * P] = 1.0
    idx = np.arange(R)
    qq = np.arange(NJ)
    pp = np.arange(P)
    in_maps = []
    for d in range(NCORES):
        # local chunk q, partition p  ->  global row j; slot order is the
        # sender id, so block q>>3 is global block q>>3 on every core.
        # Within a block the payload layout is [p, c] = row c*128 + p
        # (what the PE transpose of the deg row produces).
        jmap = ((d ^ (qq >> 3))[None, :] * R + (qq & 7)[None, :] * P
                + pp[:, None])                    # [128, 64]
        AT = np.ascontiguousarray(A[d * R:(d + 1) * R, :].T)  # [8192, 1024]
        AT[d * R + idx, idx] += 1.0               # fold in A_hat = A + I
        AT8 = AT.astype(ml_dtypes.float8_e4m3)
        ATP = np.ascontiguousarray(AT8[jmap, :]).reshape(P, NJ * R)
        XPd = np.ascontiguousarray(X16[jmap, :]).reshape(P, NJ * F)
        in_maps.append({"ATP": ATP, "XP": XPd, "WT": WTb, "B2": B2,
                        "IDN": IDN, "ESL": ESL})
    return in_maps


def kernel(X, A, W, b, _trace=False, _trace_cores=None):
    nc = _get_nc()
    in_maps = _prep_inputs(X, A, W, b)
    res = run_bass_kernel_spmd(
        nc, in_maps, list(range(NCORES)), trace=_trace,
        trace_cores=_trace_cores)
    out = np.concatenate(
        [res.results[d]["OUTT"].T for d in range(NCORES)], axis=0)
    if _trace:
        kernel.last_exec_time_ns = res.exec_time_ns
        kernel.last_results = res
    return out.astype(np.float32)


if __name__ == "__main__":
    rng = np.random.default_rng(0)
    X = rng.uniform(size=(N, F)).astype(np.float32)
    A = rng.uniform(size=(N, N)).astype(np.float32)
    W = (rng.uniform(size=(F, F)).astype(np.float32) - 0.5) / 8.0
    b = (rng.uniform(size=(F,)).astype(np.float32) - 0.5) / 8.0
    out = kernel(X, A, W, b)
    A_hat = A + np.eye(N, dtype=np.float32)
    d = 1.0 / np.sqrt(A_hat.sum(1))
    ref = (A_hat * d[:, None] * d[None, :]) @ X @ W.T + b
    err = np.abs(out - ref).max() / np.abs(ref).max()
    print("max rel err vs ref-scale:", err)
